# revision 1
# baseline (speedup 1.0000x reference)
"""CompactBilinearPooling kernel for Trainium2 (8 NeuronCores, SPMD data-parallel).

Per core (32 batch rows):
  1. Count-sketch both inputs into one fused DRAM table Y[bin] = [y1 | y2]
     (64 f32 per row): PE-transposes of x chunks -> [d, b] layout, s1 scaling
     fused into the PSUM evacuation, then NR rounds of dma_scatter_add where
     round r carries the r-th occurrence of each bin (collision-free per
     round; masked entries are scatter-added into a trash row).
  2. Circular convolution via FFT packing trick: Z = FFT(y1 + i*y2),
     out = Im(IFFT(Z^2))/2. Length-16384 FFT = 128x128 four-step with DFT-128
     matmuls on the PE in float32r (full rate). Twiddle complex-multiplies are
     decomposed: the 4 elementwise products run on DVE/GPSIMD, the +- recombine
     is absorbed into the following matmul stage as extra PSUM-accumulated
     matmuls (saves 4 DVE passes per group). Square runs on ACT + DVE.
"""
import sys

sys.path.insert(0, "/opt/trn_rl_repo")

import numpy as np

import concourse.bass as bass
import concourse.bacc as bacc
import concourse.mybir as mybir
import concourse.tile as tile
from concourse.bass_utils import run_bass_kernel_spmd
P = 128
B, D, O = 256, 4096, 16384
NCORES = 8
BC = B // NCORES          # 32 rows per core
NT = D // P               # 32 column-chunks of x
YROWS = O + 256           # trash rows O..O+127, expanded rows O+128..O+255
EXPBASE = O + 128
TRASH = O
F32R = mybir.dt.float32r
F32 = mybir.dt.float32

_cache = {}
PER_GROUP_OUT = True
M4_GPSIMD = True
TWO_LEVEL = False


def _build(n_rounds: int, skip_scatter=False, skip_fft=False):
    nc = bacc.Bacc("TRN2", target_bir_lowering=False, debug=False)

    # ---- I/O ----
    x1c = nc.dram_tensor("x1c", [BC, D], F32R, kind="ExternalInput")
    x2c = nc.dram_tensor("x2c", [BC, D], F32R, kind="ExternalInput")
    sTb = nc.dram_tensor("sTb", [P, NT * 64], F32R, kind="ExternalInput")
    idxs = nc.dram_tensor("idxs", [P, n_rounds * (D // 16)], mybir.dt.int16, kind="ExternalInput")
    idxs3 = nc.dram_tensor("idxs3", [P, 8], mybir.dt.int16, kind="ExternalInput")
    wa1 = nc.dram_tensor("wa1", [P, 2 * P], F32R, kind="ExternalInput")    # [WFre | WFim]
    wa2 = nc.dram_tensor("wa2", [P, 2 * P], F32R, kind="ExternalInput")    # [-WFim | WFre]
    wfre = nc.dram_tensor("wfre", [P, P], F32R, kind="ExternalInput")
    wfim = nc.dram_tensor("wfim", [P, P], F32R, kind="ExternalInput")
    wfimn = nc.dram_tensor("wfimn", [P, P], F32R, kind="ExternalInput")    # -WFim
    wi1 = nc.dram_tensor("wi1", [P, 2 * P], F32R, kind="ExternalInput")    # [WIre | WIim]
    wi2 = nc.dram_tensor("wi2", [P, 2 * P], F32R, kind="ExternalInput")    # [-2WIim | 2WIre]
    wire = nc.dram_tensor("wire", [P, P], F32R, kind="ExternalInput")
    wiim = nc.dram_tensor("wiim", [P, P], F32R, kind="ExternalInput")
    t1re = nc.dram_tensor("t1re", [P, 4 * P], F32R, kind="ExternalInput")   # bcast over 4 rows
    t1im = nc.dram_tensor("t1im", [P, 4 * P], F32R, kind="ExternalInput")
    t1imn = nc.dram_tensor("t1imn", [P, 4 * P], F32R, kind="ExternalInput")
    t2re = nc.dram_tensor("t2re", [P, 4 * P], F32R, kind="ExternalInput")   # x 1/(2N)
    t2im = nc.dram_tensor("t2im", [P, 4 * P], F32R, kind="ExternalInput")
    t2imn = nc.dram_tensor("t2imn", [P, 4 * P], F32R, kind="ExternalInput")
    identm = nc.dram_tensor("identm", [BC, BC], F32R, kind="ExternalInput")
    out = nc.dram_tensor("out", [BC, O], F32, kind="ExternalOutput")

    with tile.TileContext(nc) as tc:
        with (
            tc.tile_pool(name="const", bufs=1) as cp,
            tc.tile_pool(name="work", bufs=1) as wp,
            tc.tile_pool(name="tmp", bufs=2) as tp,
            tc.tile_pool(name="psum", bufs=4, space="PSUM") as pp,
            tc.tile_pool(name="dram", bufs=1, space="DRAM") as dp,
        ):
            # ---- fused sketch table in DRAM: row = [y1(32) | y2(32)] ----
            yd = dp.tile([YROWS, 64], F32R)

            # x loads + sketch zero-init first (HWDGE FIFO order = priority)
            xs1 = wp.tile([BC, D], F32R, tag="xs1_y")
            xs2 = wp.tile([BC, D], F32R, tag="xs2_s")
            nc.sync.dma_start(xs1[:], x1c[:])
            nc.sync.dma_start(xs2[:], x2c[:])
            zsb = wp.tile([P, YROWS * 32 // P], F32, tag="zero_osb")
            nc.vector.memset(zsb[:], 0.0)
            ydv = yd[:].rearrange("(h p a) e -> h p a e", h=2, p=P)
            for h in range(2):
                nc.sync.dma_start(ydv[h], zsb[:].bitcast(F32R).rearrange("p (a e) -> p a e", e=64))

            def cload(src, shape, dtype=F32R):
                t = cp.tile(shape, dtype, tag=src.name)
                nc.sync.dma_start(t[:], src[:])
                return t

            sTb_s = cload(sTb, [P, NT * 64])
            idxs_s = cp.tile([P, n_rounds * (D // 16)], mybir.dt.int16)
            nc.sync.dma_start(idxs_s[:], idxs[:])
            idxs3_s = cp.tile([P, 8], mybir.dt.int16)
            nc.sync.dma_start(idxs3_s[:], idxs3[:])
            wa1_s = cload(wa1, [P, 2 * P])
            wa2_s = cload(wa2, [P, 2 * P])
            wfre_s = cload(wfre, [P, P])
            wfim_s = cload(wfim, [P, P])
            wfimn_s = cload(wfimn, [P, P])
            wi1_s = cload(wi1, [P, 2 * P])
            wi2_s = cload(wi2, [P, 2 * P])
            wire_s = cload(wire, [P, P])
            wiim_s = cload(wiim, [P, P])
            t1re_s = cload(t1re, [P, 4 * P])
            t1im_s = cload(t1im, [P, 4 * P])
            t1imn_s = cload(t1imn, [P, 4 * P])
            t2re_s = cload(t2re, [P, 4 * P])
            t2im_s = cload(t2im, [P, 4 * P])
            t2imn_s = cload(t2imn, [P, 4 * P])
            ident_t = cload(identm, [BC, BC])
            ident = ident_t[:]


            sxT = wp.tile([P, NT * 64], F32R, tag="sxT")  # [d%128, (chunk, [x1|x2])]
            for g in range(2):  # 2 psum groups of 16 chunks
                ps = pp.tile([P, 1024], F32R, space="PSUM", tag="ps")
                for jj in range(16):
                    j = g * 16 + jj
                    nc.tensor.transpose(out=ps[:, jj * 64:jj * 64 + 32],
                                        in_=xs1[:, j * P:(j + 1) * P], identity=ident)
                    nc.tensor.transpose(out=ps[:, jj * 64 + 32:jj * 64 + 64],
                                        in_=xs2[:, j * P:(j + 1) * P], identity=ident)
                nc.vector.tensor_mul(sxT[:, g * 1024:(g + 1) * 1024], ps[:],
                                     sTb_s[:, g * 1024:(g + 1) * 1024])

            # ---- scatter rounds (dma_scatter_add; masked -> spread trash rows)
            # round 0: rank-0 entries; round 1: rank-1 at bins + rank>=2 at
            # expanded rows; a mini-scatter then folds the expanded rows into
            # their bins via an SBUF bounce.
            inap = sxT[:].rearrange("p (t e) -> p t e", e=64)
            for r in ([] if skip_scatter else range(n_rounds)):
                nc.gpsimd.dma_scatter_add(
                    out_ap=yd[:],
                    in_ap=inap,
                    idxs_ap=idxs_s[:, r * (D // 16):(r + 1) * (D // 16)],
                    num_idxs=D,
                    num_idxs_reg=D,
                    elem_size=64,
                )
            if n_rounds == 2 and not skip_scatter:
                bounce = tp.tile([P, 64], F32R, tag="m1")
                nc.sync.dma_start(bounce[:], yd[EXPBASE:EXPBASE + P, :])
                nc.gpsimd.dma_scatter_add(
                    out_ap=yd[:],
                    in_ap=bounce[:].rearrange("p (t e) -> p t e", e=64),
                    idxs_ap=idxs3_s[:],
                    num_idxs=P,
                    num_idxs_reg=P,
                    elem_size=64,
                )

            # ---- reload fused sketch as [q, (n2, 64)] ----
            yf = wp.tile([P, P * 64], F32R, tag="xs1_y")
            nc.sync.dma_start(yf[:].rearrange("q (n e) -> q n e", e=64),
                              yd[0:O, :].rearrange("(q n) e -> q n e", q=P))
            yf_r = yf[:].rearrange("q (n e) -> q n e", e=64)

            r3 = lambda ap: ap.rearrange("p (b2 k) -> p b2 k", b2=4)

            # ---- FFT: software-pipelined across 4-row groups ----
            ssb_re = wp.tile([P, P * BC], F32R, tag="ssb_re")
            ssb_im = wp.tile([P, P * BC], F32R, tag="ssb_im")
            osb = wp.tile([P, P * BC], F32, tag="zero_osb")
            mt, nt_ = {}, {}

            def stage_a(g):
                ps = pp.tile([P, 1024], F32, space="PSUM", tag="ps")
                for bb in range(4):
                    b_ = g * 4 + bb
                    sl = ps[:, bb * 256:(bb + 1) * 256]
                    nc.tensor.matmul(out=sl, lhsT=yf_r[:, :, b_], rhs=wa1_s[:], start=True, stop=False)
                    nc.tensor.matmul(out=sl, lhsT=yf_r[:, :, 32 + b_], rhs=wa2_s[:], start=False, stop=True)
                pre = ps[:].rearrange("p (b2 h k) -> p b2 h k", b2=4, h=2)[:, :, 0, :]
                pim = ps[:].rearrange("p (b2 h k) -> p b2 h k", b2=4, h=2)[:, :, 1, :]
                m1 = tp.tile([P, 512], F32R, tag="m1")
                m2 = tp.tile([P, 512], F32R, tag="m2")
                m3 = tp.tile([P, 512], F32R, tag="m3")
                m4 = tp.tile([P, 512], F32R, tag="m4")
                mim = tp.tile([P, 512], F32R, tag="m5")
                nc.scalar.copy(mim[:], pim)  # ACT evac (GPSIMD cannot read PSUM)
                nc.vector.tensor_mul(r3(m1[:]), pre, r3(t1re_s[:]))
                nc.gpsimd.tensor_mul(r3(m2[:]), r3(mim[:]), r3(t1imn_s[:]))
                nc.vector.tensor_mul(r3(m3[:]), pre, r3(t1im_s[:]))
                if M4_GPSIMD:
                    nc.gpsimd.tensor_mul(r3(m4[:]), r3(mim[:]), r3(t1re_s[:]))
                else:
                    nc.vector.tensor_mul(r3(m4[:]), pim, r3(t1re_s[:]))
                mt[g] = (m1, m2, m3, m4)

            def stage_b(g):
                m1, m2, m3, m4 = mt.pop(g)
                rs = slice(g * 512, (g + 1) * 512)
                ps = pp.tile([P, 1024], F32, space="PSUM", tag="ps")
                zre, zim = ps[:, 0:512], ps[:, 512:1024]
                nc.tensor.matmul(out=zre, lhsT=wfre_s[:], rhs=m1[:], start=True, stop=False)
                nc.tensor.matmul(out=zre, lhsT=wfre_s[:], rhs=m2[:], start=False, stop=False)
                nc.tensor.matmul(out=zre, lhsT=wfimn_s[:], rhs=m3[:], start=False, stop=False)
                nc.tensor.matmul(out=zre, lhsT=wfimn_s[:], rhs=m4[:], start=False, stop=True)
                nc.tensor.matmul(out=zim, lhsT=wfim_s[:], rhs=m1[:], start=True, stop=False)
                nc.tensor.matmul(out=zim, lhsT=wfim_s[:], rhs=m2[:], start=False, stop=False)
                nc.tensor.matmul(out=zim, lhsT=wfre_s[:], rhs=m3[:], start=False, stop=False)
                nc.tensor.matmul(out=zim, lhsT=wfre_s[:], rhs=m4[:], start=False, stop=True)
                u = tp.tile([P, 512], F32R, tag="m1")
                v = tp.tile([P, 512], F32R, tag="m2")
                w_ = tp.tile([P, 512], F32R, tag="m3")
                nc.scalar.activation(u[:], zre, mybir.ActivationFunctionType.Square)
                nc.scalar.activation(v[:], zim, mybir.ActivationFunctionType.Square)
                nc.scalar.copy(w_[:], zim)
                nc.vector.tensor_sub(ssb_re[:, rs], u[:], v[:])
                nc.vector.tensor_mul(ssb_im[:, rs], zre, w_[:])

            def stage_c(g):
                ps = pp.tile([P, 1024], F32, space="PSUM", tag="ps")
                for bb in range(4):
                    b_ = g * 4 + bb
                    sl = ps[:, bb * 256:(bb + 1) * 256]
                    lre = ssb_re[:, b_ * P:(b_ + 1) * P]
                    lim = ssb_im[:, b_ * P:(b_ + 1) * P]
                    nc.tensor.matmul(out=sl, lhsT=lre, rhs=wi1_s[:], start=True, stop=False)
                    nc.tensor.matmul(out=sl, lhsT=lim, rhs=wi2_s[:], start=False, stop=True)
                preC = ps[:].rearrange("p (b2 h k) -> p b2 h k", b2=4, h=2)[:, :, 0, :]
                pimC = ps[:].rearrange("p (b2 h k) -> p b2 h k", b2=4, h=2)[:, :, 1, :]
                n1 = tp.tile([P, 512], F32R, tag="n1")
                n2 = tp.tile([P, 512], F32R, tag="n2")
                n3 = tp.tile([P, 512], F32R, tag="n3")
                n4 = tp.tile([P, 512], F32R, tag="n4")
                nimC = tp.tile([P, 512], F32R, tag="n5")
                nc.scalar.copy(nimC[:], pimC)
                nc.vector.tensor_mul(r3(n1[:]), preC, r3(t2re_s[:]))
                nc.gpsimd.tensor_mul(r3(n2[:]), r3(nimC[:]), r3(t2imn_s[:]))
                nc.vector.tensor_mul(r3(n3[:]), preC, r3(t2im_s[:]))
                nc.vector.tensor_mul(r3(n4[:]), pimC, r3(t2re_s[:]))
                nt_[g] = (n1, n2, n3, n4)

            def stage_d(g):
                n1, n2, n3, n4 = nt_.pop(g)
                rs = slice(g * 512, (g + 1) * 512)
                ps = pp.tile([P, 1024], F32, space="PSUM", tag="ps")
                po = ps[:, 0:512]
                nc.tensor.matmul(out=po, lhsT=wiim_s[:], rhs=n1[:], start=True, stop=False)
                nc.tensor.matmul(out=po, lhsT=wiim_s[:], rhs=n2[:], start=False, stop=False)
                nc.tensor.matmul(out=po, lhsT=wire_s[:], rhs=n3[:], start=False, stop=False)
                nc.tensor.matmul(out=po, lhsT=wire_s[:], rhs=n4[:], start=False, stop=True)
                nc.scalar.copy(osb[:, rs], po)
                if PER_GROUP_OUT:
                    nc.sync.dma_start(
                        out[:].rearrange("b (a c) -> a b c", c=P)[:, g * 4:(g + 1) * 4, :],
                        osb[:, rs].rearrange("a (b c) -> a b c", c=P))

            for gg in range(11):
                if gg < 8 and not skip_fft:
                    stage_a(gg)
                if 1 <= gg < 9 and not skip_fft:
                    stage_b(gg - 1)
                if 2 <= gg < 10 and not skip_fft:
                    stage_c(gg - 2)
                if 3 <= gg and not skip_fft:
                    stage_d(gg - 3)
            if skip_fft:
                nc.vector.memset(osb[:], 0.0)
            if not PER_GROUP_OUT or skip_fft:
                nc.sync.dma_start(out[:].rearrange("b (a c) -> a b c", c=P),
                                  osb[:].rearrange("a (b c) -> a b c", c=P))


    nc.compile()
    return nc


def _host_consts():
    j = np.arange(P)
    f32 = np.float32
    ang = -2.0 * np.pi * np.outer(j, j) / P
    wf_re, wf_im = np.cos(ang), np.sin(ang)
    wi_re, wi_im = np.cos(-ang), np.sin(-ang)
    wa1 = np.concatenate([wf_re, wf_im], axis=1).astype(f32)
    wa2 = np.concatenate([-wf_im, wf_re], axis=1).astype(f32)
    wi1 = np.concatenate([wi_re, wi_im], axis=1).astype(f32)
    wi2 = np.concatenate([-2.0 * wi_im, 2.0 * wi_re], axis=1).astype(f32)
    tang = -2.0 * np.pi * np.outer(j, j) / O
    t1re_1 = np.cos(tang)
    t1im_1 = np.sin(tang)
    scale = 1.0 / (2.0 * O)
    t2re_1 = np.cos(tang) * scale      # cos(+x) = cos(-x)
    t2im_1 = -np.sin(tang) * scale     # sin(+x) = -sin(-x)

    def b4(m):
        return np.tile(m[:, None, :], (1, 4, 1)).reshape(P, 4 * P).astype(f32)

    return dict(
        wa1=wa1, wa2=wa2, wi1=wi1, wi2=wi2,
        wfre=wf_re.astype(f32), wfim=wf_im.astype(f32), wfimn=(-wf_im).astype(f32),
        wire=wi_re.astype(f32), wiim=wi_im.astype(f32),
        t1re=b4(t1re_1), t1im=b4(t1im_1), t1imn=b4(-t1im_1),
        t2re=b4(t2re_1), t2im=b4(t2im_1), t2imn=b4(-t2im_1),
        identm=np.eye(BC, dtype=f32),
    )


def _host_prep(h1, s1):
    """Per-round int16 index tables (wrapped layout) + s broadcast table."""
    h1 = np.asarray(h1, dtype=np.int64)
    s1 = np.asarray(s1, dtype=np.float32)
    rank = np.zeros(D, np.int64)
    seen = {}
    for d in range(D):
        b = int(h1[d])
        rank[d] = seen.get(b, 0)
        seen[b] = int(rank[d]) + 1
    n_hi = int((rank >= 2).sum())
    trash = (TRASH + (np.arange(D) % 128)).astype(np.int64)
    idxs3 = np.full(P, TRASH, np.int64) + np.arange(P) % 128
    if TWO_LEVEL and int(rank.max()) >= 2 and n_hi <= P:
        # two-level: round0 = rank0, round1 = rank1 + rank>=2 at expanded rows,
        # mini-scatter (idxs3) folds expanded rows into bins
        n_rounds = 2
        flat0 = np.where(rank == 0, h1, trash)
        flat1 = np.where(rank == 1, h1, trash)
        hi = np.where(rank >= 2)[0]
        for j, d in enumerate(hi):
            flat1[d] = EXPBASE + j
            idxs3[j] = h1[d]
        rounds = [flat0, flat1]
    else:
        n_rounds = int(rank.max()) + 1
        rounds = [np.where(rank == r, h1, trash) for r in range(n_rounds)]
    idxs = np.zeros((P, n_rounds * (D // 16)), np.int16)
    for r, flat in enumerate(rounds):
        wrapped = flat.astype(np.int16).reshape(D // 16, 16).T  # idx i at [i%16, i//16]
        idxs[:, r * (D // 16):(r + 1) * (D // 16)] = np.tile(wrapped, (8, 1))
    idxs3_w = np.tile(idxs3.astype(np.int16).reshape(8, 16).T, (8, 1))  # [128, 8]
    sTb = np.zeros((P, NT * 64), np.float32)
    for t in range(NT):
        sTb[:, t * 64:(t + 1) * 64] = s1[t * P:(t + 1) * P][:, None]
    return n_rounds, idxs, idxs3_w, sTb


_last_results = None


def kernel(x1, x2, h1, s1, output_size=O, **kw):
    global _last_results
    x1 = np.asarray(x1, np.float32)
    x2 = np.asarray(x2, np.float32)
    n_rounds, idxs, idxs3, sTb = _host_prep(h1, s1)
    if n_rounds not in _cache:
        _cache[n_rounds] = _build(n_rounds)
    nc = _cache[n_rounds]
    consts = _host_consts()
    in_maps = []
    for c in range(NCORES):
        m = dict(consts)
        m["x1c"] = x1[c * BC:(c + 1) * BC]
        m["x2c"] = x2[c * BC:(c + 1) * BC]
        m["idxs"] = idxs
        m["idxs3"] = idxs3
        m["sTb"] = sTb
        in_maps.append(m)
    res = run_bass_kernel_spmd(nc, in_maps, core_ids=list(range(NCORES)))
    _last_results = res
    return np.concatenate([res.results[c]["out"] for c in range(NCORES)], axis=0)



# revision 4
# speedup vs baseline: 1.4706x; 1.4706x over previous
"""CompactBilinearPooling kernel for Trainium2 (8 NeuronCores, SPMD data-parallel).

Per core (32 batch rows):
  1. Count-sketch both inputs into one fused DRAM table Y[bin] = [y1 | y2]
     (64 f32 per row). The host permutes x columns by collision rank
     (rank-r = r-th occurrence of a bin) and pads each rank class to a
     128-column chunk boundary, so scatter round r is a dma_scatter_add
     over a small chunk-aligned window whose valid tokens are exactly the
     rank-r entries (trailing -1 indices are trimmed by the Q7 ucode
     before descriptor generation). This cuts GPSIMD desc-gen from
     n_rounds*D tokens to D tokens total.
  2. Circular convolution via FFT packing trick: Z = FFT(y1 + i*y2),
     out = Im(IFFT(Z^2))/2. Length-16384 FFT = 128x128 four-step with
     DFT-128 matmuls on the PE in float32r. Twiddle complex-multiplies
     run on DVE/GPSIMD with the +/- recombine also on DVE/GPSIMD, so the
     row-DFT stages need half the matmuls (PE is the FFT bottleneck).
"""
import sys

sys.path.insert(0, "/opt/trn_rl_repo")

import numpy as np

import concourse.bass as bass
import concourse.bacc as bacc
import concourse.mybir as mybir
import concourse.tile as tile
from concourse.bass_utils import run_bass_kernel_spmd

P = 128
B, D, O = 256, 4096, 16384
NCORES = 8
BC = B // NCORES          # 32 rows per core
BC2 = 2 * BC              # x1|x2 stacked on partitions
F32R = mybir.dt.float32r
F32 = mybir.dt.float32

# packed FFT-const column offsets
_CON = {}
_off = 0
for _name, _w in [("wa1", 2 * P), ("wa2", 2 * P), ("wfre", P), ("wfim", P),
                  ("wfimn", P), ("wi1", 2 * P), ("wi2", 2 * P), ("wire", P),
                  ("wiim", P), ("t1re", 4 * P), ("t1im", 4 * P),
                  ("t2re", 4 * P), ("t2im", 4 * P)]:
    _CON[_name] = (_off, _off + _w)
    _off += _w
NCON = _off

_cache = {}


def _build(rounds):
    """rounds: tuple of (n_chunks, count) per scatter round."""
    nt = sum(nch for nch, _ in rounds)
    dp_cols = nt * P
    icols = sum(nch * 8 for nch, _ in rounds)

    nc = bacc.Bacc("TRN2", target_bir_lowering=False, debug=False)

    x12 = nc.dram_tensor("x12", [BC2, dp_cols], F32R, kind="ExternalInput")
    identm = nc.dram_tensor("identm", [BC2, BC2], F32R, kind="ExternalInput")
    idxs = nc.dram_tensor("idxs", [P, icols], mybir.dt.int16, kind="ExternalInput")
    sTb = nc.dram_tensor("sTb", [P, nt * 64], F32R, kind="ExternalInput")
    fftc = nc.dram_tensor("fftc", [P, NCON], F32R, kind="ExternalInput")
    out = nc.dram_tensor("out", [BC, O], F32, kind="ExternalOutput")

    with tile.TileContext(nc) as tc:
        with (
            tc.tile_pool(name="const", bufs=1) as cp,
            tc.tile_pool(name="work", bufs=1) as wp,
            tc.tile_pool(name="tmp", bufs=2) as tp,
            tc.tile_pool(name="psum", bufs=4, space="PSUM") as pp,
            tc.tile_pool(name="dram", bufs=1, space="DRAM") as dp,
        ):
            yd = dp.tile([O, 64], F32R)

            # ---- loads in priority order (sync queue is FIFO) ----
            xs12 = wp.tile([BC2, dp_cols], F32R, tag="xs12")
            nc.sync.dma_start(xs12[:], x12[:])
            ident_t = cp.tile([BC2, BC2], F32R, tag="ident")
            nc.sync.dma_start(ident_t[:], identm[:])
            idxs_s = cp.tile([P, icols], mybir.dt.int16, tag="idxs")
            nc.sync.dma_start(idxs_s[:], idxs[:])
            sTb_s = cp.tile([P, nt * 64], F32R, tag="sTb")
            nc.sync.dma_start(sTb_s[:], sTb[:])

            # sketch-table zero-init (issued on scalar engine; transfers
            # overlap the transpose/scatter-desc-gen phase)
            zsb = wp.tile([P, O * 64 // (2 * P)], F32, tag="zsb")
            nc.vector.memset(zsb[:], 0.0)
            ydv = yd[:].rearrange("(h p a) e -> h p a e", h=2, p=P)
            for h in range(2):
                nc.scalar.dma_start(ydv[h], zsb[:].bitcast(F32R).rearrange(
                    "p (a e) -> p a e", e=64))

            # FFT consts: one packed load, last in the sync-queue FIFO
            fftc_s = cp.tile([P, NCON], F32R, tag="fftc")
            nc.sync.dma_start(fftc_s[:], fftc[:])

            def con(name):
                a, b = _CON[name]
                return fftc_s[:, a:b]

            wa1_s, wa2_s = con("wa1"), con("wa2")
            wfre_s, wfim_s, wfimn_s = con("wfre"), con("wfim"), con("wfimn")
            wi1_s, wi2_s = con("wi1"), con("wi2")
            wire_s, wiim_s = con("wire"), con("wiim")
            t1re_s, t1im_s = con("t1re"), con("t1im")
            t2re_s, t2im_s = con("t2re"), con("t2im")
            ident = ident_t[:]

            # ---- PE transposes -> sxT [d%128, (chunk, [y1|y2] rows)] ----
            sxT = wp.tile([P, nt * 64], F32R, tag="sxT")
            groups = [(g * 16, min(16, nt - g * 16)) for g in range((nt + 15) // 16)]
            for g0, gn in groups:
                ps = pp.tile([P, 1024], F32R, space="PSUM", tag="ps")
                for jj in range(gn):
                    j = g0 + jj
                    nc.tensor.transpose(out=ps[:, jj * 64:(jj + 1) * 64],
                                        in_=xs12[:, j * P:(j + 1) * P],
                                        identity=ident)
                nc.vector.tensor_mul(sxT[:, g0 * 64:(g0 + gn) * 64],
                                     ps[:, :gn * 64],
                                     sTb_s[:, g0 * 64:(g0 + gn) * 64])

            # ---- scatter rounds: window r = rank-r tokens (chunk-aligned,
            # -1 suffix trimmed by ucode) ----
            cs, ioff = 0, 0
            for nch, cnt in rounds:
                win = nch * P
                nc.gpsimd.dma_scatter_add(
                    out_ap=yd[:],
                    in_ap=sxT[:, cs * 64:(cs + nch) * 64].rearrange(
                        "p (t e) -> p t e", e=64),
                    idxs_ap=idxs_s[:, ioff:ioff + win // 16],
                    num_idxs=win,
                    num_idxs_reg=cnt,
                    elem_size=64,
                )
                cs += nch
                ioff += win // 16

            # ---- reload fused sketch as [q, (n, e)] ----
            yf = wp.tile([P, P * 64], F32R, tag="yf")
            nc.sync.dma_start(yf[:].rearrange("q (n e) -> q n e", e=64),
                              yd[:].rearrange("(q n) e -> q n e", q=P))
            yf_r = yf[:].rearrange("q (n e) -> q n e", e=64)

            r3 = lambda ap: ap.rearrange("p (b2 k) -> p b2 k", b2=4)

            # ---- FFT: software-pipelined across 4-row groups ----
            ssb_re = wp.tile([P, P * BC], F32R, tag="ssb_re")
            ssb_im = wp.tile([P, P * BC], F32R, tag="ssb_im")
            osb = wp.tile([P, P * BC], F32, tag="osb")
            mt, nt_ = {}, {}

            def stage_a(g):
                ps = pp.tile([P, 1024], F32, space="PSUM", tag="ps")
                for bb in range(4):
                    b_ = g * 4 + bb
                    sl = ps[:, bb * 256:(bb + 1) * 256]
                    nc.tensor.matmul(out=sl, lhsT=yf_r[:, :, b_], rhs=wa1_s,
                                     start=True, stop=False)
                    nc.tensor.matmul(out=sl, lhsT=yf_r[:, :, 32 + b_], rhs=wa2_s,
                                     start=False, stop=True)
                pre = ps[:].rearrange("p (b2 h k) -> p b2 h k", b2=4, h=2)[:, :, 0, :]
                pim = ps[:].rearrange("p (b2 h k) -> p b2 h k", b2=4, h=2)[:, :, 1, :]
                mim = tp.tile([P, 512], F32R, tag="m5")
                a1 = tp.tile([P, 512], F32R, tag="m1")
                a2 = tp.tile([P, 512], F32R, tag="m2")
                a3 = tp.tile([P, 512], F32R, tag="m3")
                a4 = tp.tile([P, 512], F32R, tag="m4")
                mre = tp.tile([P, 512], F32R, tag="mre")
                mim2 = tp.tile([P, 512], F32R, tag="mim2")
                nc.scalar.copy(mim[:], pim)  # ACT evac (GPSIMD cannot read PSUM)
                nc.vector.tensor_mul(r3(a1[:]), pre, r3(t1re_s))
                nc.gpsimd.tensor_mul(r3(a2[:]), r3(mim[:]), r3(t1im_s))
                nc.vector.tensor_mul(r3(a3[:]), pre, r3(t1im_s))
                nc.gpsimd.tensor_mul(r3(a4[:]), r3(mim[:]), r3(t1re_s))
                nc.vector.tensor_sub(mre[:], a1[:], a2[:])
                nc.gpsimd.tensor_add(mim2[:], a3[:], a4[:])
                mt[g] = (mre, mim2)

            def stage_b(g):
                mre, mim2 = mt.pop(g)
                rs = slice(g * 512, (g + 1) * 512)
                ps = pp.tile([P, 1024], F32, space="PSUM", tag="ps")
                zre, zim = ps[:, 0:512], ps[:, 512:1024]
                nc.tensor.matmul(out=zre, lhsT=wfre_s, rhs=mre[:], start=True, stop=False)
                nc.tensor.matmul(out=zre, lhsT=wfimn_s, rhs=mim2[:], start=False, stop=True)
                nc.tensor.matmul(out=zim, lhsT=wfim_s, rhs=mre[:], start=True, stop=False)
                nc.tensor.matmul(out=zim, lhsT=wfre_s, rhs=mim2[:], start=False, stop=True)
                u = tp.tile([P, 512], F32R, tag="m1")
                v = tp.tile([P, 512], F32R, tag="m2")
                w_ = tp.tile([P, 512], F32R, tag="m3")
                nc.scalar.activation(u[:], zre, mybir.ActivationFunctionType.Square)
                nc.scalar.activation(v[:], zim, mybir.ActivationFunctionType.Square)
                nc.scalar.copy(w_[:], zim)
                nc.vector.tensor_sub(ssb_re[:, rs], u[:], v[:])
                nc.vector.tensor_mul(ssb_im[:, rs], zre, w_[:])

            def stage_c(g):
                ps = pp.tile([P, 1024], F32, space="PSUM", tag="ps")
                for bb in range(4):
                    b_ = g * 4 + bb
                    sl = ps[:, bb * 256:(bb + 1) * 256]
                    lre = ssb_re[:, b_ * P:(b_ + 1) * P]
                    lim = ssb_im[:, b_ * P:(b_ + 1) * P]
                    nc.tensor.matmul(out=sl, lhsT=lre, rhs=wi1_s, start=True, stop=False)
                    nc.tensor.matmul(out=sl, lhsT=lim, rhs=wi2_s, start=False, stop=True)
                preC = ps[:].rearrange("p (b2 h k) -> p b2 h k", b2=4, h=2)[:, :, 0, :]
                pimC = ps[:].rearrange("p (b2 h k) -> p b2 h k", b2=4, h=2)[:, :, 1, :]
                nimC = tp.tile([P, 512], F32R, tag="n5")
                c1 = tp.tile([P, 512], F32R, tag="n1")
                c2 = tp.tile([P, 512], F32R, tag="n2")
                c3 = tp.tile([P, 512], F32R, tag="n3")
                c4 = tp.tile([P, 512], F32R, tag="n4")
                nre = tp.tile([P, 512], F32R, tag="nre")
                nim2 = tp.tile([P, 512], F32R, tag="nim2")
                nc.scalar.copy(nimC[:], pimC)
                nc.vector.tensor_mul(r3(c1[:]), preC, r3(t2re_s))
                nc.gpsimd.tensor_mul(r3(c2[:]), r3(nimC[:]), r3(t2im_s))
                nc.vector.tensor_mul(r3(c3[:]), preC, r3(t2im_s))
                nc.gpsimd.tensor_mul(r3(c4[:]), r3(nimC[:]), r3(t2re_s))
                nc.vector.tensor_sub(nre[:], c1[:], c2[:])
                nc.vector.tensor_add(nim2[:], c3[:], c4[:])
                nt_[g] = (nre, nim2)

            def stage_d(g):
                nre, nim2 = nt_.pop(g)
                rs = slice(g * 512, (g + 1) * 512)
                ps = pp.tile([P, 1024], F32, space="PSUM", tag="ps")
                po = ps[:, 0:512]
                nc.tensor.matmul(out=po, lhsT=wiim_s, rhs=nre[:], start=True, stop=False)
                nc.tensor.matmul(out=po, lhsT=wire_s, rhs=nim2[:], start=False, stop=True)
                nc.scalar.copy(osb[:, rs], po)
                nc.sync.dma_start(
                    out[:].rearrange("b (a c) -> a b c", c=P)[:, g * 4:(g + 1) * 4, :],
                    osb[:, rs].rearrange("a (b c) -> a b c", c=P))

            for gg in range(11):
                if gg < 8:
                    stage_a(gg)
                if 1 <= gg < 9:
                    stage_b(gg - 1)
                if 2 <= gg < 10:
                    stage_c(gg - 2)
                if 3 <= gg:
                    stage_d(gg - 3)

    nc.compile()
    return nc


def _host_consts():
    j = np.arange(P)
    f32 = np.float32
    ang = -2.0 * np.pi * np.outer(j, j) / P
    wf_re, wf_im = np.cos(ang), np.sin(ang)
    wi_re, wi_im = np.cos(-ang), np.sin(-ang)
    tang = -2.0 * np.pi * np.outer(j, j) / O
    scale = 1.0 / (2.0 * O)

    def b4(m):
        return np.tile(m[:, None, :], (1, 4, 1)).reshape(P, 4 * P)

    parts = {
        "wa1": np.concatenate([wf_re, wf_im], axis=1),
        "wa2": np.concatenate([-wf_im, wf_re], axis=1),
        "wfre": wf_re, "wfim": wf_im, "wfimn": -wf_im,
        "wi1": np.concatenate([wi_re, wi_im], axis=1),
        "wi2": np.concatenate([-2.0 * wi_im, 2.0 * wi_re], axis=1),
        "wire": wi_re, "wiim": wi_im,
        "t1re": b4(np.cos(tang)), "t1im": b4(np.sin(tang)),
        "t2re": b4(np.cos(tang) * scale), "t2im": b4(-np.sin(tang) * scale),
    }
    fftc = np.zeros((P, NCON), f32)
    for name, (a, b) in _CON.items():
        fftc[:, a:b] = parts[name]
    return dict(fftc=fftc, identm=np.eye(BC2, dtype=f32))


def _host_prep(h1, s1):
    """Rank-sorted, chunk-padded column layout + per-round idx tables."""
    h1 = np.asarray(h1, dtype=np.int64)
    s1 = np.asarray(s1, dtype=np.float32)
    rank = np.zeros(D, np.int64)
    seen = {}
    for d in range(D):
        b = int(h1[d])
        rank[d] = seen.get(b, 0)
        seen[b] = int(rank[d]) + 1
    nr = int(rank.max()) + 1
    order = np.argsort(rank, kind="stable")
    counts = [int((rank == r).sum()) for r in range(nr)]

    rounds = []
    src = []          # source d per padded position (-1 = pad)
    flat = []         # concatenated idx tables (position-within-window)
    pos = 0
    for r in range(nr):
        cnt = counts[r]
        if cnt == 0:
            continue
        nch = (cnt + P - 1) // P
        win = nch * P
        cls = order[pos:pos + cnt]
        src.extend(cls.tolist())
        src.extend([-1] * (win - cnt))
        f = np.full(win, -1, np.int64)
        f[:cnt] = h1[cls]
        flat.append(f)
        rounds.append((nch, cnt))
        pos += cnt
    src = np.asarray(src, np.int64)
    dp_cols = src.shape[0]
    nt = dp_cols // P

    # wrapped idx layout: idx i at [i%16, i//16], replicated to 128 partitions
    idxs = np.concatenate(
        [np.tile(f.astype(np.int16).reshape(-1, 16).T, (8, 1)) for f in flat],
        axis=1)

    s_pad = np.zeros(dp_cols, np.float32)
    valid = src >= 0
    s_pad[valid] = s1[src[valid]]
    # sTb[p, c*64+e] = s_pad[c*128+p]
    sTb = np.ascontiguousarray(
        np.broadcast_to(s_pad.reshape(nt, P).T[:, :, None], (P, nt, 64))
    ).reshape(P, nt * 64)
    return tuple(rounds), src, idxs, sTb


_last_results = None


def kernel(x1, x2, h1, s1, output_size=O, **kw):
    global _last_results
    x1 = np.asarray(x1, np.float32)
    x2 = np.asarray(x2, np.float32)
    rounds, src, idxs, sTb = _host_prep(h1, s1)
    if rounds not in _cache:
        _cache[rounds] = _build(rounds)
    nc = _cache[rounds]
    consts = _host_consts()
    dp_cols = src.shape[0]
    valid = src >= 0
    in_maps = []
    for c in range(NCORES):
        m = dict(consts)
        x12 = np.zeros((BC2, dp_cols), np.float32)
        x12[:BC, valid] = x1[c * BC:(c + 1) * BC][:, src[valid]]
        x12[BC:, valid] = x2[c * BC:(c + 1) * BC][:, src[valid]]
        m["x12"] = x12
        m["idxs"] = idxs
        m["sTb"] = sTb
        in_maps.append(m)
    res = run_bass_kernel_spmd(nc, in_maps, core_ids=list(range(NCORES)))
    _last_results = res
    return np.concatenate([res.results[c]["out"] for c in range(NCORES)], axis=0)


# revision 8
# speedup vs baseline: 1.8415x; 1.2522x over previous
"""CompactBilinearPooling kernel for Trainium2 (8 NeuronCores, SPMD data-parallel).

Per core (32 batch rows):
  1. Count-sketch both inputs into a parity-split SBUF table pair
     (A = even bin%128, B = odd) via dma_scatter_add's SBUF-dst CCE-add
     mode with idx' = (bin%128)*128 + bin//128, so partition = bin//128
     and the FFT reads the table in place (no DRAM bounce / reload).
     The host permutes x columns by collision rank and pads each rank
     class to a 128-column chunk, so scatter round r covers exactly the
     rank-r tokens (trailing -1 idxs are trimmed by the Q7 ucode before
     desc-gen). Rank>=2 classes (<=128 tokens) are pre-folded into their
     rank-1 partner columns with partition-aligned DVE adds (pad columns
     are zero), leaving 2 scatter rounds. The whole sketch path is fp16
     (halves DMA/CCE bytes; |y| <= ~10 so range is safe).
  2. Circular convolution via FFT packing: Z = FFT(y1 + i*y2),
     out = Im(IFFT(Z^2))/2. Length-16384 FFT = 128x128 four-step with
     DFT-128 matmuls on the PE (fp16 in, f32 PSUM accumulate). A
     1/sqrt(2*16384) scale in the first DFT replaces the 1/(2N) ifft
     normalization so every intermediate fits fp16. PSUM results are
     ACT-evacuated to flat fp16 tiles so all twiddle/recombine ops run
     as dense 16-bit DVE/GPSIMD tensor_tensor (2x uop). The parity
     split is a fixed permutation pi of the inner FFT digit, absorbed
     host-side into twiddle/weight row order.
"""
import sys

sys.path.insert(0, "/opt/trn_rl_repo")

import numpy as np

import concourse.bass as bass
import concourse.bacc as bacc
import concourse.mybir as mybir
import concourse.tile as tile
from concourse.bass_utils import run_bass_kernel_spmd

P = 128
B, D, O = 256, 4096, 16384
NCORES = 8
BC = B // NCORES          # 32 rows per core
BC2 = 2 * BC              # x1|x2 stacked on partitions
F32R = mybir.dt.float32r
F32 = mybir.dt.float32
F16 = mybir.dt.float16

# packed fp16 consts
_CON = {}
_off = 0
for _name, _w in [("wa1", 2 * P), ("wa2", 2 * P), ("wfre", P), ("wfim", P),
                  ("wfimn", P), ("wi1", 2 * P), ("wi2", 2 * P), ("wire", P),
                  ("wiim", P), ("t1re", 4 * P), ("t1im", 4 * P),
                  ("t2re", 4 * P), ("t2im", 4 * P)]:
    _CON[_name] = (_off, _off + _w)
    _off += _w
NCON = _off

_cache = {}


def _build(rounds, folds, nt):
    """rounds: ((n_chunks, count), ...) scatter rounds.
    folds: ((src_chunk, dst_chunk), ...) payload folds, applied in order.
    nt: total chunks in the padded layout."""
    icols = sum(nch * 8 for nch, _ in rounds)

    nc = bacc.Bacc("TRN2", target_bir_lowering=False, debug=False)

    x12 = nc.dram_tensor("x12", [BC2, nt * P], F16, kind="ExternalInput")
    identm = nc.dram_tensor("identm", [BC2, BC2], F16, kind="ExternalInput")
    idxs = nc.dram_tensor("idxs", [P, icols], mybir.dt.int16, kind="ExternalInput")
    sTb = nc.dram_tensor("sTb", [P, nt * 64], F16, kind="ExternalInput")
    fftc = nc.dram_tensor("fftc", [P, NCON], F16, kind="ExternalInput")
    out = nc.dram_tensor("out", [BC, O], F32, kind="ExternalOutput")

    with tile.TileContext(nc) as tc:
        with (
            tc.tile_pool(name="const", bufs=1) as cp,
            tc.tile_pool(name="work", bufs=1) as wp,
            tc.tile_pool(name="tmp", bufs=2) as tp,
            tc.tile_pool(name="psum", bufs=4, space="PSUM") as pp,
        ):
            # ---- loads, priority order on one FIFO queue ----
            xs12 = wp.tile([BC2, nt * P], F16, tag="xs12")
            nc.sync.dma_start(xs12[:], x12[:])
            ident_t = cp.tile([BC2, BC2], F16, tag="ident")
            nc.sync.dma_start(ident_t[:], identm[:])
            idxs_s = cp.tile([P, icols], mybir.dt.int16, tag="idxs")
            nc.sync.dma_start(idxs_s[:], idxs[:])
            sTb_s = cp.tile([P, nt * 64], F16, tag="sTb")
            nc.sync.dma_start(sTb_s[:], sTb[:])
            fftc_s = cp.tile([P, NCON], F16, tag="fftc")
            nc.sync.dma_start(fftc_s[:], fftc[:])

            def con(name):
                a, b = _CON[name]
                return fftc_s[:, a:b]

            wa1_s, wa2_s = con("wa1"), con("wa2")
            wfre_s, wfim_s, wfimn_s = con("wfre"), con("wfim"), con("wfimn")
            wi1_s, wi2_s = con("wi1"), con("wi2")
            wire_s, wiim_s = con("wire"), con("wiim")
            t1re_s, t1im_s = con("t1re"), con("t1im")
            t2re_s, t2im_s = con("t2re"), con("t2im")
            ident = ident_t[:]

            # ---- parity-split fp16 sketch table in SBUF ----
            y2 = wp.tile([P, 2 * 64 * 32], F32, tag="y2")   # fp16 pair view
            nc.vector.memset(y2[:, 0:2048], 0.0)
            nc.gpsimd.memset(y2[:, 2048:4096], 0.0)
            tabA = y2[:, 0:2048].bitcast(F16)
            tabB = y2[:, 2048:4096].bitcast(F16)

            # ---- PE transposes -> sxT [d%128, (chunk, [y1|y2] rows)] ----
            sxT = wp.tile([P, nt * 64], F16, tag="sxT")
            groups = [(g * 16, min(16, nt - g * 16)) for g in range((nt + 15) // 16)]
            for g0, gn in groups:
                ps = pp.tile([P, 1024], F16, space="PSUM", tag="psd", bufs=2)
                for jj in range(gn):
                    j = g0 + jj
                    nc.tensor.transpose(out=ps[:, jj * 64:(jj + 1) * 64],
                                        in_=xs12[:, j * P:(j + 1) * P],
                                        identity=ident)
                nc.vector.tensor_mul(sxT[:, g0 * 64:(g0 + gn) * 64],
                                     ps[:, :gn * 64],
                                     sTb_s[:, g0 * 64:(g0 + gn) * 64])

            # ---- fold rank>=2 payloads into their rank-1 partner columns ----
            for src_c, dst_c in folds:
                nc.vector.tensor_add(sxT[:, dst_c * 64:(dst_c + 1) * 64],
                                     sxT[:, dst_c * 64:(dst_c + 1) * 64],
                                     sxT[:, src_c * 64:(src_c + 1) * 64])

            # ---- scatter rounds (SBUF-dst parity-split CCE add, fp16) ----
            cs, ioff = 0, 0
            for nch, cnt in rounds:
                win = nch * P
                nc.gpsimd.dma_scatter_add(
                    out_ap=tabA,
                    out_ap_other=tabB,
                    in_ap=sxT[:, cs * 64:(cs + nch) * 64].rearrange(
                        "p (t e) -> p t e", e=64),
                    idxs_ap=idxs_s[:, ioff:ioff + win // 16],
                    num_idxs=win,
                    num_idxs_reg=cnt,
                    elem_size=64,
                    sbuf_tokens_per_rank=P,
                    parity_reg=0,
                )
                cs += nch
                ioff += win // 16

            # ---- FFT reads the table in place: lhsT [q, (par, g)] where
            # (par, g) linearizes to pi(n) = (n%2)*64 + n//2 ----
            y2r = y2[:].bitcast(F16).rearrange("q (par g e) -> q par g e",
                                               par=2, e=64)

            ssb_re = wp.tile([P, P * BC], F16, tag="ssb_re")
            ssb_im = wp.tile([P, P * BC], F16, tag="ssb_im")
            osb = wp.tile([P, P * BC], F32, tag="osb")
            mt, nt_ = {}, {}

            def stage_a(g):
                ps = pp.tile([P, 1024], F32, space="PSUM", tag="ps", bufs=3)
                for bb in range(4):
                    b_ = g * 4 + bb
                    sl = ps[:, bb * 256:(bb + 1) * 256]
                    nc.tensor.matmul(out=sl, lhsT=y2r[:, :, :, b_], rhs=wa1_s,
                                     start=True, stop=False)
                    nc.tensor.matmul(out=sl, lhsT=y2r[:, :, :, 32 + b_], rhs=wa2_s,
                                     start=False, stop=True)
                pre = ps[:].rearrange("p (b2 h k) -> p b2 h k", b2=4, h=2)[:, :, 0, :]
                pim = ps[:].rearrange("p (b2 h k) -> p b2 h k", b2=4, h=2)[:, :, 1, :]
                pres = tp.tile([P, 512], F16, tag="m6")
                pims = tp.tile([P, 512], F16, tag="m5")
                a1 = tp.tile([P, 512], F16, tag="m1")
                a2 = tp.tile([P, 512], F16, tag="m2")
                a3 = tp.tile([P, 512], F16, tag="m3")
                a4 = tp.tile([P, 512], F16, tag="m4")
                mre = tp.tile([P, 512], F16, tag="mre")
                mim2 = tp.tile([P, 512], F16, tag="mim2")
                nc.scalar.copy(pres[:].rearrange("p (b2 k) -> p b2 k", b2=4), pre)
                nc.scalar.copy(pims[:].rearrange("p (b2 k) -> p b2 k", b2=4), pim)
                nc.vector.tensor_mul(a1[:], pres[:], t1re_s)
                nc.gpsimd.tensor_mul(a2[:], pims[:], t1im_s)
                nc.vector.tensor_mul(a3[:], pres[:], t1im_s)
                nc.gpsimd.tensor_mul(a4[:], pims[:], t1re_s)
                nc.vector.tensor_sub(mre[:], a1[:], a2[:])
                nc.vector.tensor_add(mim2[:], a3[:], a4[:])
                mt[g] = (mre, mim2)

            def stage_b(g):
                mre, mim2 = mt.pop(g)
                rs = slice(g * 512, (g + 1) * 512)
                ps = pp.tile([P, 1024], F32, space="PSUM", tag="ps", bufs=3)
                zre, zim = ps[:, 0:512], ps[:, 512:1024]
                nc.tensor.matmul(out=zre, lhsT=wfre_s, rhs=mre[:], start=True, stop=False)
                nc.tensor.matmul(out=zre, lhsT=wfimn_s, rhs=mim2[:], start=False, stop=True)
                nc.tensor.matmul(out=zim, lhsT=wfim_s, rhs=mre[:], start=True, stop=False)
                nc.tensor.matmul(out=zim, lhsT=wfre_s, rhs=mim2[:], start=False, stop=True)
                zres = tp.tile([P, 512], F16, tag="m1")
                zims = tp.tile([P, 512], F16, tag="m2")
                sp = tp.tile([P, 512], F16, tag="m3")
                sm = tp.tile([P, 512], F16, tag="m4")
                nc.scalar.copy(zres[:], zre)
                nc.scalar.copy(zims[:], zim)
                nc.vector.tensor_add(sp[:], zres[:], zims[:])
                nc.vector.tensor_sub(sm[:], zres[:], zims[:])
                nc.vector.tensor_mul(ssb_re[:, rs], sp[:], sm[:])
                nc.vector.tensor_mul(ssb_im[:, rs], zres[:], zims[:])

            def stage_c(g):
                ps = pp.tile([P, 1024], F32, space="PSUM", tag="ps", bufs=3)
                for bb in range(4):
                    b_ = g * 4 + bb
                    sl = ps[:, bb * 256:(bb + 1) * 256]
                    lre = ssb_re[:, b_ * P:(b_ + 1) * P]
                    lim = ssb_im[:, b_ * P:(b_ + 1) * P]
                    nc.tensor.matmul(out=sl, lhsT=lre, rhs=wi1_s, start=True, stop=False)
                    nc.tensor.matmul(out=sl, lhsT=lim, rhs=wi2_s, start=False, stop=True)
                preC = ps[:].rearrange("p (b2 h k) -> p b2 h k", b2=4, h=2)[:, :, 0, :]
                pimC = ps[:].rearrange("p (b2 h k) -> p b2 h k", b2=4, h=2)[:, :, 1, :]
                preCs = tp.tile([P, 512], F16, tag="n6")
                pimCs = tp.tile([P, 512], F16, tag="n5")
                c1 = tp.tile([P, 512], F16, tag="n1")
                c2 = tp.tile([P, 512], F16, tag="n2")
                c3 = tp.tile([P, 512], F16, tag="n3")
                c4 = tp.tile([P, 512], F16, tag="n4")
                nre = tp.tile([P, 512], F16, tag="nre")
                nim2 = tp.tile([P, 512], F16, tag="nim2")
                nc.scalar.copy(preCs[:].rearrange("p (b2 k) -> p b2 k", b2=4), preC)
                nc.scalar.copy(pimCs[:].rearrange("p (b2 k) -> p b2 k", b2=4), pimC)
                nc.vector.tensor_mul(c1[:], preCs[:], t2re_s)
                nc.gpsimd.tensor_mul(c2[:], pimCs[:], t2im_s)
                nc.vector.tensor_mul(c3[:], preCs[:], t2im_s)
                nc.gpsimd.tensor_mul(c4[:], pimCs[:], t2re_s)
                nc.vector.tensor_sub(nre[:], c1[:], c2[:])
                nc.vector.tensor_add(nim2[:], c3[:], c4[:])
                nt_[g] = (nre, nim2)

            def stage_d(g):
                nre, nim2 = nt_.pop(g)
                rs = slice(g * 512, (g + 1) * 512)
                ps = pp.tile([P, 512], F32, space="PSUM", tag="psd", bufs=2)
                po = ps[:, 0:512]
                nc.tensor.matmul(out=po, lhsT=wiim_s, rhs=nre[:], start=True, stop=False)
                nc.tensor.matmul(out=po, lhsT=wire_s, rhs=nim2[:], start=False, stop=True)
                nc.scalar.copy(osb[:, rs], po)
                nc.sync.dma_start(
                    out[:].rearrange("b (a c) -> a b c", c=P)[:, g * 4:(g + 1) * 4, :],
                    osb[:, rs].rearrange("a (b c) -> a b c", c=P))

            for gg in range(11):
                if gg < 8:
                    stage_a(gg)
                if 1 <= gg < 9:
                    stage_b(gg - 1)
                if 2 <= gg < 10:
                    stage_c(gg - 2)
                if 3 <= gg:
                    stage_d(gg - 3)

    nc.compile()
    return nc


# pi(n) = (n%2)*64 + n//2 is the table's inner-digit order; row p of a
# permuted matrix holds the row for n = inv_pi(p) = 2*(p%64) + p//64
_PI_INV = np.array([2 * (p % 64) + p // 64 for p in range(P)])


def _host_consts():
    j = np.arange(P)
    ang = -2.0 * np.pi * np.outer(j, j) / P
    wf_re, wf_im = np.cos(ang), np.sin(ang)
    wi_re, wi_im = np.cos(-ang), np.sin(-ang)
    tang = -2.0 * np.pi * np.outer(j, j) / O
    alpha = 1.0 / np.sqrt(2.0 * O)   # replaces the 1/(2N) ifft normalization

    def b4(m):
        return np.tile(m[:, None, :], (1, 4, 1)).reshape(P, 4 * P)

    parts = {
        "wa1": np.concatenate([wf_re, wf_im], axis=1) * alpha,
        "wa2": np.concatenate([-wf_im, wf_re], axis=1) * alpha,
        "wfre": wf_re[_PI_INV], "wfim": wf_im[_PI_INV], "wfimn": -wf_im[_PI_INV],
        "wi1": np.concatenate([wi_re, wi_im], axis=1),
        "wi2": np.concatenate([-2.0 * wi_im, 2.0 * wi_re], axis=1),
        "wire": wi_re, "wiim": wi_im,
        "t1re": b4(np.cos(tang)[_PI_INV]), "t1im": b4(np.sin(tang)[_PI_INV]),
        "t2re": b4(np.cos(tang)), "t2im": b4(-np.sin(tang)),
    }
    fftc = np.zeros((P, NCON), np.float16)
    for name, (a, b) in _CON.items():
        fftc[:, a:b] = parts[name].astype(np.float16)
    return dict(fftc=fftc, identm=np.eye(BC2, dtype=np.float16))


def _host_prep(h1, s1):
    """Rank-sorted, chunk-padded column layout; rank>=2 classes of <=128
    tokens are folded (device-side adds) into rank-1 partner columns."""
    h1 = np.asarray(h1, dtype=np.int64)
    s1 = np.asarray(s1, dtype=np.float32)
    rank = np.zeros(D, np.int64)
    seen = {}
    for d in range(D):
        b = int(h1[d])
        rank[d] = seen.get(b, 0)
        seen[b] = int(rank[d]) + 1
    nr = int(rank.max()) + 1
    order = np.argsort(rank, kind="stable")
    starts = np.concatenate([[0], np.cumsum([int((rank == r).sum())
                                             for r in range(nr)])])
    classes = [order[starts[r]:starts[r + 1]] for r in range(nr)]
    # order class r so that partners of class r+1 tokens come first, in
    # class r+1's (final) order -- enables the partition-aligned fold
    for r in range(nr - 2, 0, -1):
        nxt = classes[r + 1]
        if len(nxt) == 0:
            continue
        pos_in_next = {int(h1[d]): i for i, d in enumerate(nxt)}
        keyed = sorted(range(len(classes[r])),
                       key=lambda i: pos_in_next.get(int(h1[classes[r][i]]),
                                                     1 << 30))
        classes[r] = classes[r][keyed]
    foldable = nr <= 2 or all(len(classes[r]) <= P for r in range(2, nr))

    rounds, flat, src = [], [], []
    start_chunk = {}
    chunk = 0
    for r in range(nr):
        cls = classes[r]
        cnt = len(cls)
        if cnt == 0:
            continue
        nch = (cnt + P - 1) // P
        win = nch * P
        start_chunk[r] = chunk
        src.extend(cls.tolist())
        src.extend([-1] * (win - cnt))
        if not (r >= 2 and foldable):
            f = np.full(win, -1, np.int64)
            f[:cnt] = (h1[cls] % P) * P + h1[cls] // P   # sigma(bin)
            flat.append(f)
            rounds.append((nch, cnt))
        chunk += nch
    # fold deepest class first: class r's chunk adds into class r-1's first
    fold_pairs = []
    if foldable:
        for r in range(nr - 1, 1, -1):
            if r in start_chunk:
                fold_pairs.append((start_chunk[r], start_chunk[r - 1]))
    src = np.asarray(src, np.int64)
    dp_cols = src.shape[0]
    nt = dp_cols // P

    idxs = np.concatenate(
        [np.tile(f.astype(np.int16).reshape(-1, 16).T, (8, 1)) for f in flat],
        axis=1)

    s_pad = np.zeros(dp_cols, np.float32)
    valid = src >= 0
    s_pad[valid] = s1[src[valid]]
    # sTb[p, c*64+e] = s_pad[c*128+p]
    sTb = np.ascontiguousarray(
        np.broadcast_to(s_pad.reshape(nt, P).T[:, :, None], (P, nt, 64))
    ).reshape(P, nt * 64).astype(np.float16)
    return tuple(rounds), tuple(fold_pairs), nt, src, idxs, sTb


_last_results = None


def kernel(x1, x2, h1, s1, output_size=O, **kw):
    global _last_results
    x1 = np.asarray(x1, np.float32)
    x2 = np.asarray(x2, np.float32)
    rounds, folds, nt, src, idxs, sTb = _host_prep(h1, s1)
    key = (rounds, folds, nt)
    if key not in _cache:
        _cache[key] = _build(rounds, folds, nt)
    nc = _cache[key]
    consts = _host_consts()
    valid = src >= 0
    in_maps = []
    for c in range(NCORES):
        m = dict(consts)
        x12 = np.zeros((BC2, nt * P), np.float16)
        x12[:BC, valid] = x1[c * BC:(c + 1) * BC][:, src[valid]]
        x12[BC:, valid] = x2[c * BC:(c + 1) * BC][:, src[valid]]
        m["x12"] = x12
        m["idxs"] = idxs
        m["sTb"] = sTb
        in_maps.append(m)
    res = run_bass_kernel_spmd(nc, in_maps, core_ids=list(range(NCORES)))
    _last_results = res
    return np.concatenate([res.results[c]["out"] for c in range(NCORES)], axis=0)


# revision 10
# speedup vs baseline: 2.3695x; 1.2867x over previous
"""CompactBilinearPooling kernel for Trainium2 (8 NeuronCores, SPMD data-parallel).

Per core (32 batch rows):
  1. Count-sketch both inputs into a parity-split fp16 SBUF table pair
     (A = even bin%128, B = odd) via dma_scatter_add's SBUF-dst CCE-add
     mode with idx' = (bin%128)*128 + bin//128, so partition = bin//128
     and the FFT reads the table in place. The host sorts x columns by
     collision rank (padded to 128-col chunks) and orders each rank
     class so class r+1's token at window position j has its same-bin
     class-r partner at position j; partition-aligned DVE chunk adds
     then fold every rank>=1 payload down into the rank-0 columns (pad
     columns are zero), leaving ONE scatter instruction whose tokens all
     hit distinct bins (~8.5us fixed Q7 dispatch + ~7ns/token desc-gen).
  2. Circular convolution via FFT packing: Z = FFT(y1 + i*y2),
     out = Im(IFFT(Z^2))/2. Length-16384 FFT = 128x128 four-step, fp16
     in / f32 PSUM matmuls, with 1/sqrt(2*16384) folded into the first
     DFT so all intermediates fit fp16. Stage outputs are evacuated by
     ACT into flat fp16 tiles (re/im h-major in PSUM via strided matmul
     outs), and ALL twiddle/recombine/square elementwise ops run on DVE
     as dense 16-bit tensor_tensor (2x uop), fused across group PAIRS
     ([128,1024] ops) to amortize the per-op overhead. GPSIMD does no
     FFT work: concurrent DVE+GPSIMD SBUF traffic was measured to slow
     both ~2.3x. The parity split is a fixed permutation pi of the
     inner FFT digit, absorbed host-side into twiddle/weight row order.
"""
import sys

sys.path.insert(0, "/opt/trn_rl_repo")

import numpy as np

import concourse.bass as bass
import concourse.bacc as bacc
import concourse.mybir as mybir
import concourse.tile as tile
from concourse.bass_utils import run_bass_kernel_spmd

P = 128
B, D, O = 256, 4096, 16384
NCORES = 8
BC = B // NCORES          # 32 rows per core
BC2 = 2 * BC              # x1|x2 stacked on partitions
F32R = mybir.dt.float32r
F32 = mybir.dt.float32
F16 = mybir.dt.float16

# packed fp16 consts (t-tables pair-tiled to 8*P)
_CON = {}
_off = 0
for _name, _w in [("wa1", 2 * P), ("wa2", 2 * P), ("wfre", P), ("wfim", P),
                  ("wfimn", P), ("wi1", 2 * P), ("wi2", 2 * P), ("wire", P),
                  ("wiim", P), ("t1re", 8 * P), ("t1im", 8 * P),
                  ("t2re", 8 * P), ("t2im", 8 * P)]:
    _CON[_name] = (_off, _off + _w)
    _off += _w
NCON = _off

_cache = {}


def _build(rounds, folds, nt):
    """rounds: ((n_chunks, count), ...) scatter rounds (usually one).
    folds: ((src_chunk, dst_chunk), ...) payload folds, applied in order.
    nt: total chunks in the padded layout."""
    icols = sum(nch * 8 for nch, _ in rounds)

    nc = bacc.Bacc("TRN2", target_bir_lowering=False, debug=False)

    x12 = nc.dram_tensor("x12", [BC2, nt * P], F16, kind="ExternalInput")
    identm = nc.dram_tensor("identm", [BC2, BC2], F16, kind="ExternalInput")
    idxs = nc.dram_tensor("idxs", [P, icols], mybir.dt.int16, kind="ExternalInput")
    sTb = nc.dram_tensor("sTb", [P, nt * 64], F16, kind="ExternalInput")
    fftc = nc.dram_tensor("fftc", [P, NCON], F16, kind="ExternalInput")
    out = nc.dram_tensor("out", [BC, O], F32, kind="ExternalOutput")

    with tile.TileContext(nc) as tc:
        with (
            tc.tile_pool(name="const", bufs=1) as cp,
            tc.tile_pool(name="work", bufs=1) as wp,
            tc.tile_pool(name="tmp", bufs=2) as tp,
            tc.tile_pool(name="psum", bufs=4, space="PSUM") as pp,
        ):
            # ---- loads, priority order on one FIFO queue ----
            xs12 = wp.tile([BC2, nt * P], F16, tag="xs12")
            nc.sync.dma_start(xs12[:], x12[:])
            ident_t = cp.tile([BC2, BC2], F16, tag="ident")
            nc.sync.dma_start(ident_t[:], identm[:])
            idxs_s = cp.tile([P, icols], mybir.dt.int16, tag="idxs")
            nc.sync.dma_start(idxs_s[:], idxs[:])
            sTb_s = cp.tile([P, nt * 64], F16, tag="sTb")
            nc.sync.dma_start(sTb_s[:], sTb[:])
            fftc_s = cp.tile([P, NCON], F16, tag="fftc")
            nc.sync.dma_start(fftc_s[:], fftc[:])

            def con(name):
                a, b = _CON[name]
                return fftc_s[:, a:b]

            wa1_s, wa2_s = con("wa1"), con("wa2")
            wfre_s, wfim_s, wfimn_s = con("wfre"), con("wfim"), con("wfimn")
            wi1_s, wi2_s = con("wi1"), con("wi2")
            wire_s, wiim_s = con("wire"), con("wiim")
            t1re_s, t1im_s = con("t1re"), con("t1im")
            t2re_s, t2im_s = con("t2re"), con("t2im")
            ident = ident_t[:]

            # ---- parity-split fp16 sketch table in SBUF ----
            y2 = wp.tile([P, 2 * 64 * 32], F32, tag="y2")   # fp16 pair view
            nc.vector.memset(y2[:, 0:2048], 0.0)
            nc.gpsimd.memset(y2[:, 2048:4096], 0.0)
            tabA = y2[:, 0:2048].bitcast(F16)
            tabB = y2[:, 2048:4096].bitcast(F16)

            # ---- PE transposes -> sxT [d%128, (chunk, [y1|y2] rows)] ----
            sxT = wp.tile([P, nt * 64], F16, tag="sxT")
            groups = [(g * 16, min(16, nt - g * 16)) for g in range((nt + 15) // 16)]
            for g0, gn in groups:
                ps = pp.tile([P, 1024], F16, space="PSUM", tag="psd", bufs=2)
                for jj in range(gn):
                    j = g0 + jj
                    nc.tensor.transpose(out=ps[:, jj * 64:(jj + 1) * 64],
                                        in_=xs12[:, j * P:(j + 1) * P],
                                        identity=ident)
                nc.vector.tensor_mul(sxT[:, g0 * 64:(g0 + gn) * 64],
                                     ps[:, :gn * 64],
                                     sTb_s[:, g0 * 64:(g0 + gn) * 64])

            # ---- fold rank>=1 payloads down into rank-0 partner columns ----
            for src_c, dst_c in folds:
                nc.vector.tensor_add(sxT[:, dst_c * 64:(dst_c + 1) * 64],
                                     sxT[:, dst_c * 64:(dst_c + 1) * 64],
                                     sxT[:, src_c * 64:(src_c + 1) * 64])

            # ---- scatter (SBUF-dst parity-split CCE add, fp16) ----
            cs, ioff = 0, 0
            for nch, cnt in rounds:
                win = nch * P
                nc.gpsimd.dma_scatter_add(
                    out_ap=tabA,
                    out_ap_other=tabB,
                    in_ap=sxT[:, cs * 64:(cs + nch) * 64].rearrange(
                        "p (t e) -> p t e", e=64),
                    idxs_ap=idxs_s[:, ioff:ioff + win // 16],
                    num_idxs=win,
                    num_idxs_reg=cnt,
                    elem_size=64,
                    sbuf_tokens_per_rank=P,
                    parity_reg=0,
                )
                cs += nch
                ioff += win // 16

            # ---- FFT reads the table in place: lhsT [q, (par, g)] where
            # (par, g) linearizes to pi(n) = (n%2)*64 + n//2 ----
            y2r = y2[:].bitcast(F16).rearrange("q (par g e) -> q par g e",
                                               par=2, e=64)

            ssb_re = wp.tile([P, P * BC], F16, tag="ssb_re")
            ssb_im = wp.tile([P, P * BC], F16, tag="ssb_im")
            osb = wp.tile([P, P * BC], F32, tag="osb")
            mts, nts = {}, {}

            def halves(ps):
                v = ps[:].rearrange("p (b2 h k) -> p b2 h k", b2=4, h=2)
                return v[:, :, 0, :], v[:, :, 1, :]

            def pair_a(pg):
                presP = tp.tile([P, 1024], F16, tag="presP")
                pimsP = tp.tile([P, 1024], F16, tag="pimsP")
                for gh in range(2):
                    g = 2 * pg + gh
                    ps = pp.tile([P, 1024], F32, space="PSUM", tag="ps", bufs=3)
                    for bb in range(4):
                        b_ = g * 4 + bb
                        sl = ps[:, bb * 256:(bb + 1) * 256]
                        nc.tensor.matmul(out=sl, lhsT=y2r[:, :, :, b_],
                                         rhs=wa1_s, start=True, stop=False)
                        nc.tensor.matmul(out=sl, lhsT=y2r[:, :, :, 32 + b_],
                                         rhs=wa2_s, start=False, stop=True)
                    pre, pim = halves(ps)
                    hs = slice(gh * 512, (gh + 1) * 512)
                    r3h = lambda t: t[:, hs].rearrange("p (b2 k) -> p b2 k", b2=4)
                    nc.scalar.copy(r3h(presP), pre)
                    nc.scalar.copy(r3h(pimsP), pim)
                a1 = tp.tile([P, 1024], F16, tag="m1")
                a2 = tp.tile([P, 1024], F16, tag="m2")
                a3 = tp.tile([P, 1024], F16, tag="m3")
                a4 = tp.tile([P, 1024], F16, tag="m4")
                mreP = tp.tile([P, 1024], F16, tag="mreP")
                mim2P = tp.tile([P, 1024], F16, tag="mim2P")
                nc.vector.tensor_mul(a1[:], presP[:], t1re_s)
                nc.vector.tensor_mul(a2[:], pimsP[:], t1im_s)
                nc.vector.tensor_mul(a3[:], presP[:], t1im_s)
                nc.vector.tensor_mul(a4[:], pimsP[:], t1re_s)
                nc.vector.tensor_sub(mreP[:], a1[:], a2[:])
                nc.vector.tensor_add(mim2P[:], a3[:], a4[:])
                mts[pg] = (mreP, mim2P)

            def pair_b(pg):
                mreP, mim2P = mts.pop(pg)
                zresP = tp.tile([P, 1024], F16, tag="zresP")
                zimsP = tp.tile([P, 1024], F16, tag="zimsP")
                for gh in range(2):
                    hs = slice(gh * 512, (gh + 1) * 512)
                    ps = pp.tile([P, 1024], F32, space="PSUM", tag="ps", bufs=3)
                    zre, zim = ps[:, 0:512], ps[:, 512:1024]
                    nc.tensor.matmul(out=zre, lhsT=wfre_s, rhs=mreP[:, hs],
                                     start=True, stop=False)
                    nc.tensor.matmul(out=zre, lhsT=wfimn_s, rhs=mim2P[:, hs],
                                     start=False, stop=True)
                    nc.tensor.matmul(out=zim, lhsT=wfim_s, rhs=mreP[:, hs],
                                     start=True, stop=False)
                    nc.tensor.matmul(out=zim, lhsT=wfre_s, rhs=mim2P[:, hs],
                                     start=False, stop=True)
                    nc.scalar.copy(zresP[:, hs], zre)
                    nc.scalar.copy(zimsP[:, hs], zim)
                rs = slice(pg * 1024, (pg + 1) * 1024)
                sp = tp.tile([P, 1024], F16, tag="m1")
                sm = tp.tile([P, 1024], F16, tag="m2")
                nc.vector.tensor_add(sp[:], zresP[:], zimsP[:])
                nc.vector.tensor_sub(sm[:], zresP[:], zimsP[:])
                nc.vector.tensor_mul(ssb_re[:, rs], sp[:], sm[:])
                nc.vector.tensor_mul(ssb_im[:, rs], zresP[:], zimsP[:])

            def pair_c(pg):
                preCsP = tp.tile([P, 1024], F16, tag="preCsP")
                pimCsP = tp.tile([P, 1024], F16, tag="pimCsP")
                for gh in range(2):
                    g = 2 * pg + gh
                    ps = pp.tile([P, 1024], F32, space="PSUM", tag="ps", bufs=3)
                    for bb in range(4):
                        b_ = g * 4 + bb
                        sl = ps[:, bb * 256:(bb + 1) * 256]
                        lre = ssb_re[:, b_ * P:(b_ + 1) * P]
                        lim = ssb_im[:, b_ * P:(b_ + 1) * P]
                        nc.tensor.matmul(out=sl, lhsT=lre, rhs=wi1_s,
                                         start=True, stop=False)
                        nc.tensor.matmul(out=sl, lhsT=lim, rhs=wi2_s,
                                         start=False, stop=True)
                    preC, pimC = halves(ps)
                    hs = slice(gh * 512, (gh + 1) * 512)
                    r3h = lambda t: t[:, hs].rearrange("p (b2 k) -> p b2 k", b2=4)
                    nc.scalar.copy(r3h(preCsP), preC)
                    nc.scalar.copy(r3h(pimCsP), pimC)
                c1 = tp.tile([P, 1024], F16, tag="n1")
                c2 = tp.tile([P, 1024], F16, tag="n2")
                c3 = tp.tile([P, 1024], F16, tag="n3")
                c4 = tp.tile([P, 1024], F16, tag="n4")
                nreP = tp.tile([P, 1024], F16, tag="nreP")
                nim2P = tp.tile([P, 1024], F16, tag="nim2P")
                nc.vector.tensor_mul(c1[:], preCsP[:], t2re_s)
                nc.vector.tensor_mul(c2[:], pimCsP[:], t2im_s)
                nc.vector.tensor_mul(c3[:], preCsP[:], t2im_s)
                nc.vector.tensor_mul(c4[:], pimCsP[:], t2re_s)
                nc.vector.tensor_sub(nreP[:], c1[:], c2[:])
                nc.vector.tensor_add(nim2P[:], c3[:], c4[:])
                nts[pg] = (nreP, nim2P)

            def pair_d(pg):
                nreP, nim2P = nts.pop(pg)
                for gh in range(2):
                    g = 2 * pg + gh
                    hs = slice(gh * 512, (gh + 1) * 512)
                    rs = slice(g * 512, (g + 1) * 512)
                    ps = pp.tile([P, 512], F32, space="PSUM", tag="psd", bufs=2)
                    po = ps[:, 0:512]
                    nc.tensor.matmul(out=po, lhsT=wiim_s, rhs=nreP[:, hs],
                                     start=True, stop=False)
                    nc.tensor.matmul(out=po, lhsT=wire_s, rhs=nim2P[:, hs],
                                     start=False, stop=True)
                    nc.scalar.copy(osb[:, rs], po)
                    nc.sync.dma_start(
                        out[:].rearrange("b (a c) -> a b c", c=P)[:, g * 4:(g + 1) * 4, :],
                        osb[:, rs].rearrange("a (b c) -> a b c", c=P))

            for t in range(7):
                if t < 4:
                    pair_a(t)
                if 1 <= t < 5:
                    pair_b(t - 1)
                if 2 <= t < 6:
                    pair_c(t - 2)
                if 3 <= t:
                    pair_d(t - 3)

    nc.compile()
    return nc


# pi(n) = (n%2)*64 + n//2 is the table's inner-digit order; row p of a
# permuted matrix holds the row for n = inv_pi(p) = 2*(p%64) + p//64
_PI_INV = np.array([2 * (p % 64) + p // 64 for p in range(P)])


def _host_consts():
    j = np.arange(P)
    ang = -2.0 * np.pi * np.outer(j, j) / P
    wf_re, wf_im = np.cos(ang), np.sin(ang)
    wi_re, wi_im = np.cos(-ang), np.sin(-ang)
    tang = -2.0 * np.pi * np.outer(j, j) / O
    alpha = 1.0 / np.sqrt(2.0 * O)   # replaces the 1/(2N) ifft normalization

    def b8(m):
        t = np.tile(m[:, None, :], (1, 4, 1)).reshape(P, 4 * P)
        return np.concatenate([t, t], axis=1)

    parts = {
        "wa1": np.concatenate([wf_re, wf_im], axis=1) * alpha,
        "wa2": np.concatenate([-wf_im, wf_re], axis=1) * alpha,
        "wfre": wf_re[_PI_INV], "wfim": wf_im[_PI_INV], "wfimn": -wf_im[_PI_INV],
        "wi1": np.concatenate([wi_re, wi_im], axis=1),
        "wi2": np.concatenate([-2.0 * wi_im, 2.0 * wi_re], axis=1),
        "wire": wi_re, "wiim": wi_im,
        "t1re": b8(np.cos(tang)[_PI_INV]), "t1im": b8(np.sin(tang)[_PI_INV]),
        "t2re": b8(np.cos(tang)), "t2im": b8(-np.sin(tang)),
    }
    fftc = np.zeros((P, NCON), np.float16)
    for name, (a, b) in _CON.items():
        fftc[:, a:b] = parts[name].astype(np.float16)
    return dict(fftc=fftc, identm=np.eye(BC2, dtype=np.float16))


def _host_prep(h1, s1):
    """Rank-sorted, chunk-padded column layout; every rank>=1 class is
    folded (device-side chunk adds) into its rank-(r-1) partner columns,
    leaving a single all-distinct-bins scatter."""
    h1 = np.asarray(h1, dtype=np.int64)
    s1 = np.asarray(s1, dtype=np.float32)
    rank = np.zeros(D, np.int64)
    seen = {}
    for d in range(D):
        b = int(h1[d])
        rank[d] = seen.get(b, 0)
        seen[b] = int(rank[d]) + 1
    nr = int(rank.max()) + 1
    order = np.argsort(rank, kind="stable")
    starts = np.concatenate([[0], np.cumsum([int((rank == r).sum())
                                             for r in range(nr)])])
    classes = [order[starts[r]:starts[r + 1]] for r in range(nr)]
    # order class r so that the partner of class r+1's token at window
    # position j sits at class r's position j (pads have no constraint)
    for r in range(nr - 2, -1, -1):
        nxt = classes[r + 1]
        if len(nxt) == 0:
            continue
        pos_in_next = {int(h1[d]): i for i, d in enumerate(nxt)}
        keyed = sorted(range(len(classes[r])),
                       key=lambda i: pos_in_next.get(int(h1[classes[r][i]]),
                                                     1 << 30))
        classes[r] = classes[r][keyed]
    nchs = [(len(c) + P - 1) // P for c in classes]
    foldable = all(len(classes[r]) <= len(classes[r - 1]) and
                   nchs[r] <= nchs[r - 1] for r in range(1, nr))

    rounds, flat, src = [], [], []
    start_chunk = {}
    chunk = 0
    for r in range(nr):
        cls = classes[r]
        cnt = len(cls)
        if cnt == 0:
            continue
        nch = nchs[r]
        win = nch * P
        start_chunk[r] = chunk
        src.extend(cls.tolist())
        src.extend([-1] * (win - cnt))
        if r == 0 or not foldable:
            f = np.full(win, -1, np.int64)
            f[:cnt] = (h1[cls] % P) * P + h1[cls] // P   # sigma(bin)
            flat.append(f)
            rounds.append((nch, cnt))
        chunk += nch
    # fold deepest class first; class r adds chunk-wise into class r-1's
    # first chunks (partner positions are partition/chunk aligned)
    fold_pairs = []
    if foldable:
        for r in range(nr - 1, 0, -1):
            if r not in start_chunk:
                continue
            for i in range(nchs[r]):
                fold_pairs.append((start_chunk[r] + i, start_chunk[r - 1] + i))
    src = np.asarray(src, np.int64)
    dp_cols = src.shape[0]
    nt = dp_cols // P

    idxs = np.concatenate(
        [np.tile(f.astype(np.int16).reshape(-1, 16).T, (8, 1)) for f in flat],
        axis=1)

    s_pad = np.zeros(dp_cols, np.float32)
    valid = src >= 0
    s_pad[valid] = s1[src[valid]]
    # sTb[p, c*64+e] = s_pad[c*128+p]
    sTb = np.ascontiguousarray(
        np.broadcast_to(s_pad.reshape(nt, P).T[:, :, None], (P, nt, 64))
    ).reshape(P, nt * 64).astype(np.float16)
    return tuple(rounds), tuple(fold_pairs), nt, src, idxs, sTb


_last_results = None


def kernel(x1, x2, h1, s1, output_size=O, **kw):
    global _last_results
    x1 = np.asarray(x1, np.float32)
    x2 = np.asarray(x2, np.float32)
    rounds, folds, nt, src, idxs, sTb = _host_prep(h1, s1)
    key = (rounds, folds, nt)
    if key not in _cache:
        _cache[key] = _build(rounds, folds, nt)
    nc = _cache[key]
    consts = _host_consts()
    valid = src >= 0
    in_maps = []
    for c in range(NCORES):
        m = dict(consts)
        x12 = np.zeros((BC2, nt * P), np.float16)
        x12[:BC, valid] = x1[c * BC:(c + 1) * BC][:, src[valid]]
        x12[BC:, valid] = x2[c * BC:(c + 1) * BC][:, src[valid]]
        m["x12"] = x12
        m["idxs"] = idxs
        m["sTb"] = sTb
        in_maps.append(m)
    res = run_bass_kernel_spmd(nc, in_maps, core_ids=list(range(NCORES)))
    _last_results = res
    return np.concatenate([res.results[c]["out"] for c in range(NCORES)], axis=0)
